# revision 43
# baseline (speedup 1.0000x reference)
"""Block-sparse (block-diagonal, BLOCK=64) multi-head attention for 8 Trainium2 cores.

Sharding: the B*S = 4096 token rows are split into 8 contiguous slices of 512
tokens (attention is block-diagonal with 64-token blocks, so slices at
512-token boundaries are fully independent). Each core runs the whole
projections + attention + output projection for its 512 tokens; weights are
replicated. No collectives; host concatenates the per-core outputs.

Schedule v2 (phase-clustered; from trace analysis of v1 at ~103us):
  v1 interleaved (64,128)-tiled score matmuls, (128,64)-tiled rowsum/AV
  matmuls and (128,128) projection matmuls every 2-3 instructions; each PE
  array tiling-mode switch drains the tensor engine (~170ns), costing
  ~15us across the run. v2 clusters matmuls by PE tiling mode, per chunk:
    S:  16 score matmuls (2 per head pair, T0/T8 row-tiles run pairs
        concurrently), each pair into one [128,256] psum tile; ONE
        [128,256] exp per head pair (scalar), then 2 big strided memsets
        per half-chunk zero the exp(garbage) cross-block quadrants.
    P1: V projection for chunk c+1 (16 x 512-row matmuls, (128,128)),
        psum evacuated by scalar Identity activations (vector is busier).
    R:  batched rowsums: ONE (128,64)-tiled N=512 matmul per 4 head
        pairs per psum half (T0/T1 concurrent; strided rhs over the
        persistent p2 tensor), then per head pair: reciprocal (vector),
        2 AV matmuls ((128,64) T0/T1 pair), oT = AV * rec (vector).
    P2: Y projection for chunk c-1 + bias add (vector) + DMA (alternating
        HWDGE/SWDGE queues).
  Tail: the last chunk's Y projection defers only the m=7 accumulations
  past the final AV/mul; the two bias adds run on gpsimd and vector in
  parallel, each followed immediately by its output DMA.

  Kept from v1: exact algebraic simplifications (bk dropped: per-query
  constant is softmax-invariant; bv folded into bo on the host), the
  two-queue DMA lead-in ordered by first use, ~100 junk warmup matmuls
  bridging the ~13us DMA/queue-ramp window, fp32 PSUM accumulation.

Compute dtype: bf16 operands (runs measurably faster than fp16 on the PE
and the ~6e-3 rel err is well within the 2e-2 gate).
"""

import sys

sys.path.insert(0, "/opt/trn_rl_repo")

import numpy as np

N_CORES = 8
B, S, D = 2, 2048, 1024
H, DK = 16, 64
T = (B * S) // N_CORES      # 512 tokens per core
P = 128
KO = D // P                 # 8 contraction tiles
MO = D // P                 # 8 d_out tiles
NC_CHUNKS = T // P          # 4 token chunks per core
HP = H // 2                 # 8 head pairs

# PE warmup matmuls turned out to be counterproductive: they burn the HAM
# power budget during the DMA lead-in and trigger an early ~7us 50%-duty
# clamp; without them the run is consistently ~4us faster (the cold-start
# pstate ramp is absorbed by the DMA-bound early phase).
WARMUP = 0

_cache = {}


def _build_program(compute):
    import concourse.tile as tile
    from concourse import bacc, mybir

    f32 = mybir.dt.float32
    dtc = {"f32": f32, "f16": mybir.dt.float16, "bf16": mybir.dt.bfloat16}[compute]

    nc = bacc.Bacc("TRN2", target_bir_lowering=False, debug=False)

    xq_d = nc.dram_tensor("xq", [P, KO, T], dtc, kind="ExternalInput").ap()
    xk_d = nc.dram_tensor("xk", [P, KO, T], dtc, kind="ExternalInput").ap()
    xv_d = nc.dram_tensor("xv", [P, KO, T], dtc, kind="ExternalInput").ap()
    wq_d = nc.dram_tensor("wq", [MO, P, KO, P], dtc, kind="ExternalInput").ap()
    wk_d = nc.dram_tensor("wk", [MO, P, KO, P], dtc, kind="ExternalInput").ap()
    wv_d = nc.dram_tensor("wv", [D, D], dtc, kind="ExternalInput").ap()
    wo_d = nc.dram_tensor("wo", [D, D], dtc, kind="ExternalInput").ap()
    bq_d = nc.dram_tensor("bq", [P, MO], f32, kind="ExternalInput").ap()
    bo_d = nc.dram_tensor("bo", [D], dtc, kind="ExternalInput").ap()
    y_d = nc.dram_tensor("y", [T, D], dtc, kind="ExternalOutput").ap()

    with tile.TileContext(nc) as tc:
        with (
            tc.tile_pool(name="singles", bufs=1) as singles,
            tc.tile_pool(name="p2", bufs=2) as p2_pool,
            tc.tile_pool(name="rec", bufs=4) as rec_pool,
            tc.tile_pool(name="ystage", bufs=3) as y_pool,
            tc.tile_pool(name="ytail", bufs=4) as yt_pool,
            tc.tile_pool(name="psS", bufs=2, space="PSUM") as psS,
            tc.tile_pool(name="psP", bufs=2, space="PSUM") as psP,
            tc.tile_pool(name="psRS", bufs=1, space="PSUM") as psRS,
            tc.tile_pool(name="psAV", bufs=2, space="PSUM") as psAV,
        ):
            # ---- persistent SBUF tensors ----
            xq_sb = singles.tile([P, KO, T], dtc, tag="xq")
            xk_sb = singles.tile([P, KO, T], dtc, tag="xk")
            xv_sb = singles.tile([P, KO, T], dtc, tag="xv")
            wq_t = [singles.tile([P, KO, P], dtc, tag=f"wq{i}", name=f"wq{i}") for i in range(MO)]
            wk_t = [singles.tile([P, KO, P], dtc, tag=f"wk{i}", name=f"wk{i}") for i in range(MO)]
            wv_t = [singles.tile([P, D], dtc, tag=f"wv{i}", name=f"wv{i}") for i in range(KO)]
            wo_t = [singles.tile([P, D], dtc, tag=f"wo{i}", name=f"wo{i}") for i in range(KO)]
            # zero-padded per-head layouts: slot (m, j) holds head 2m+j's dk
            # on its native partition range (j=0: rows 0-63, j=1: rows
            # 64-127), other half zeroed once at startup -> score matmuls
            # contract over K=128 and stay in (128,128) PE mode.
            qT_sb = singles.tile([P, MO, 2, T], dtc, tag="qT")
            kT_sb = singles.tile([P, MO, 2, T], dtc, tag="kT")
            v_sb = singles.tile([P, NC_CHUNKS, D], dtc, tag="v")
            oT_sb = singles.tile([P, MO, T], dtc, tag="oT")
            bq_sb = singles.tile([P, MO], f32, tag="bq")
            bo_sb = singles.tile([P, D], dtc, tag="bo")
            ones_sb = singles.tile([P, P], dtc, tag="ones")
            inv128_sb = singles.tile([P, P], dtc, tag="inv128")

            # PE warm-up: junk matmuls while the DMA lead-in runs, sized to
            # end right as the first projection operands arrive
            nc.vector.memset(ones_sb[:], 1.0)
            nc.vector.memset(inv128_sb[:], 1.0 / 128.0)
            # zero the pad halves of qT/kT (vector is idle during the lead-in)
            nc.vector.memset(qT_sb[64:128, :, 0, :], 0.0)
            nc.vector.memset(qT_sb[0:64, :, 1, :], 0.0)
            nc.vector.memset(kT_sb[64:128, :, 0, :], 0.0)
            nc.vector.memset(kT_sb[0:64, :, 1, :], 0.0)
            import os
            wu_m = int(os.environ.get("WU_M", "64"))    # out partitions: power knob
            wu_n = int(os.environ.get("WU_N", str(WARMUP)))
            ps_w = psRS.tile([64, 64], f32, tag="psrs", name="warmup")
            for _ in range(wu_n):
                # lhsT reads the first xq DMA piece: the burst is DMA-gated so
                # it ramps the PE pstate right before Q-proj instead of
                # burning HAM power budget during the idle DMA lead-in
                nc.tensor.matmul(ps_w[0:wu_m, 0:64],
                                 xq_sb[:, 0, 0:wu_m], ones_sb[:, 0:64],
                                 start=True, stop=True)

            # ---- input DMAs, two queues, ordered so the first Q-proj
            # matmul's operands (xq quarter 0 + wq0) head different queues
            def xpart(sb, dr, a, b):
                return (sb[:, a:b, :].rearrange("p k t -> p (k t)"),
                        dr[:, a:b, :].rearrange("p k t -> p (k t)"))

            def wtile(t, dr):
                return (t[:].rearrange("p k c -> p (k c)"), dr)

            sq = [
                xpart(xq_sb, xq_d, 0, 2), xpart(xq_sb, xq_d, 2, 4),
                wtile(wq_t[1], wq_d[1].rearrange("p k c -> p (k c)")),
                wtile(wq_t[3], wq_d[3].rearrange("p k c -> p (k c)")),
                xpart(xk_sb, xk_d, 0, 4),
                wtile(wq_t[5], wq_d[5].rearrange("p k c -> p (k c)")),
                wtile(wq_t[7], wq_d[7].rearrange("p k c -> p (k c)")),
                wtile(wk_t[1], wk_d[1].rearrange("p k c -> p (k c)")),
                wtile(wk_t[3], wk_d[3].rearrange("p k c -> p (k c)")),
                wtile(wk_t[5], wk_d[5].rearrange("p k c -> p (k c)")),
                wtile(wk_t[7], wk_d[7].rearrange("p k c -> p (k c)")),
                xpart(xv_sb, xv_d, 0, 4),
                (wv_t[0][:], wv_d[0:P, :]), (wv_t[2][:], wv_d[2 * P:3 * P, :]),
                (wv_t[4][:], wv_d[4 * P:5 * P, :]), (wv_t[6][:], wv_d[6 * P:7 * P, :]),
                (wo_t[0][:], wo_d[0:P, :]), (wo_t[2][:], wo_d[2 * P:3 * P, :]),
                (wo_t[4][:], wo_d[4 * P:5 * P, :]), (wo_t[6][:], wo_d[6 * P:7 * P, :]),
            ]
            gq = [
                (bq_sb[:], bq_d[:]),
                wtile(wq_t[0], wq_d[0].rearrange("p k c -> p (k c)")),
                xpart(xq_sb, xq_d, 4, 8),
                wtile(wq_t[2], wq_d[2].rearrange("p k c -> p (k c)")),
                wtile(wq_t[4], wq_d[4].rearrange("p k c -> p (k c)")),
                wtile(wq_t[6], wq_d[6].rearrange("p k c -> p (k c)")),
                xpart(xk_sb, xk_d, 4, 8),
                wtile(wk_t[0], wk_d[0].rearrange("p k c -> p (k c)")),
                wtile(wk_t[2], wk_d[2].rearrange("p k c -> p (k c)")),
                wtile(wk_t[4], wk_d[4].rearrange("p k c -> p (k c)")),
                wtile(wk_t[6], wk_d[6].rearrange("p k c -> p (k c)")),
                xpart(xv_sb, xv_d, 4, 8),
                (wv_t[1][:], wv_d[P:2 * P, :]), (wv_t[3][:], wv_d[3 * P:4 * P, :]),
                (wv_t[5][:], wv_d[5 * P:6 * P, :]), (wv_t[7][:], wv_d[7 * P:8 * P, :]),
                (bo_sb[:], bo_d[None, :].to_broadcast([P, D])),
                (wo_t[1][:], wo_d[P:2 * P, :]), (wo_t[3][:], wo_d[3 * P:4 * P, :]),
                (wo_t[5][:], wo_d[5 * P:6 * P, :]), (wo_t[7][:], wo_d[7 * P:8 * P, :]),
            ]
            for dst, src in sq:
                nc.sync.dma_start(dst, src)
            for dst, src in gq:
                nc.gpsimd.dma_start(dst, src)

            # ---- Q projection (feature-major out, bias via scalar) ----
            for m in range(MO):
                ps = psP.tile([P, T], f32, tag="psP", name=f"psq{m}")
                for ko in range(KO):
                    nc.tensor.matmul(ps[:], wq_t[m][:, ko, :], xq_sb[:, ko, :],
                                     start=(ko == 0), stop=(ko == KO - 1))
                nc.scalar.activation(qT_sb[0:64, m, 0, :], ps[0:64, :],
                                     mybir.ActivationFunctionType.Identity,
                                     bias=bq_sb[0:64, m:m + 1])
                nc.scalar.activation(qT_sb[64:128, m, 1, :], ps[64:128, :],
                                     mybir.ActivationFunctionType.Identity,
                                     bias=bq_sb[64:128, m:m + 1])
            # ---- K projection (no bias: softmax-invariant) ----
            for m in range(MO):
                ps = psP.tile([P, T], f32, tag="psP", name=f"psk{m}")
                for ko in range(KO):
                    nc.tensor.matmul(ps[:], wk_t[m][:, ko, :], xk_sb[:, ko, :],
                                     start=(ko == 0), stop=(ko == KO - 1))
                nc.vector.tensor_copy(kT_sb[0:64, m, 0, :], ps[0:64, :])
                nc.vector.tensor_copy(kT_sb[64:128, m, 1, :], ps[64:128, :])

            # ---- projection-phase helpers ((128,128) PE mode) ----
            def v_half(mt, n):
                # V projection half for chunk mt
                ps = psP.tile([P, T], f32, tag="psP", name=f"psv{mt}_{n}")
                for ko in range(KO):
                    nc.tensor.matmul(ps[:],
                                     xv_sb[:, ko, mt * P:(mt + 1) * P],
                                     wv_t[ko][:, n * T:(n + 1) * T],
                                     start=(ko == 0), stop=(ko == KO - 1))
                nc.scalar.activation(v_sb[:, mt, n * T:(n + 1) * T], ps[:],
                                     mybir.ActivationFunctionType.Identity)

            def v_chunk(mt):
                v_half(mt, 0)
                v_half(mt, 1)

            def y_half(c, n):
                # Y projection half for chunk c + bias + output DMA
                ps = psP.tile([P, T], f32, tag="psP", name=f"psy{c}_{n}")
                for m in range(MO):
                    nc.tensor.matmul(ps[:], oT_sb[:, m, c * P:(c + 1) * P],
                                     wo_t[m][:, n * T:(n + 1) * T],
                                     start=(m == 0), stop=(m == MO - 1))
                y_sb = y_pool.tile([P, T], dtc, tag="ystage")
                nc.vector.tensor_add(y_sb[:], ps[:], bo_sb[:, n * T:(n + 1) * T])
                nc.sync.dma_start(y_d[c * P:(c + 1) * P, n * T:(n + 1) * T], y_sb[:])

            def y_chunk_tail(c):
                # last chunk: bias pre-added into psum via a constant matmul
                # (inv128^T @ bo_bcast = bo), m=0..6 accumulate while attention
                # finishes; only the m=7 matmuls + quarter copies + DMAs trail
                # the final oT mul. Copies run on scalar+vector in parallel.
                pss = []
                for n in range(2):
                    ps = psP.tile([P, T], f32, tag="psP", name=f"psyt_{n}")
                    pss.append(ps)
                    nc.tensor.matmul(ps[:], inv128_sb[:],
                                     bo_sb[:, n * T:(n + 1) * T],
                                     start=True, stop=False)
                    for m in range(MO - 1):
                        nc.tensor.matmul(ps[:], oT_sb[:, m, c * P:(c + 1) * P],
                                         wo_t[m][:, n * T:(n + 1) * T],
                                         start=False, stop=False)
                for n in range(2):
                    nc.tensor.matmul(pss[n][:], oT_sb[:, MO - 1, c * P:(c + 1) * P],
                                     wo_t[MO - 1][:, n * T:(n + 1) * T],
                                     start=False, stop=True)
                # halves: copies run on scalar and vector in parallel, both
                # DMAs on the sync HWDGE queue (the gpsimd SWDGE queue drains
                # ~4x slower and would add ~3us to the tail)
                for n in range(2):
                    yq = yt_pool.tile([P, T], dtc, tag="ytq", name=f"ytq{n}")
                    if n == 0:
                        nc.scalar.activation(yq[:], pss[n][:],
                                             mybir.ActivationFunctionType.Identity)
                    else:
                        nc.vector.tensor_copy(yq[:], pss[n][:])
                    nc.sync.dma_start(
                        y_d[c * P:(c + 1) * P, n * T:(n + 1) * T], yq[:])

            # persistent p2 tensor: [p, head_pair, head_in_pair, q] exp(scores)
            p2_all = p2_pool.tile([P, HP, 2, P], dtc, tag="p2")

            def scores_group(c, hps):
                # scores for 2 head pairs into one psum bank (K=128 via the
                # zero-padded layouts -> stays in (128,128) PE mode; offset
                # psum writes are fine for K=128) + one [128,256] exp each
                tsl = slice(c * P, (c + 1) * P)
                ps4 = psS.tile([P, 4 * P], f32, tag="pss", name=f"pss{hps[0] // 2}")
                for i, hp in enumerate(hps):
                    for j in range(2):
                        nc.tensor.matmul(ps4[:, (2 * i + j) * P:(2 * i + j + 1) * P],
                                         kT_sb[:, hp, j, tsl],
                                         qT_sb[:, hp, j, tsl],
                                         start=True, stop=True)
                    nc.scalar.activation(
                        p2_all[:, hp, :, :].rearrange("p a b -> p (a b)"),
                        ps4[:, 2 * i * P:(2 * i + 2) * P],
                        mybir.ActivationFunctionType.Exp, scale=0.125)
                    # zero exp(garbage) cross-block quadrants (per head pair)
                    nc.gpsimd.memset(p2_all[0:64, hp, :, 64:128], 0.0)
                    nc.gpsimd.memset(p2_all[64:128, hp, :, 0:64], 0.0)

            def r_phase(c):
                # batched rowsums (full-M ones -> replicated denominators) and
                # one N=256 AV matmul per head pair with [V_h1|V_h2] lhsT: the
                # valid outputs are the diagonal [64,128] quadrants. All
                # (128,128) PE mode: zero tiling-mode transitions in the body.
                tsl = slice(c * P, (c + 1) * P)
                for hc in range(2):
                    rs = psRS.tile([P, 4 * P], f32, tag="psrs", name=f"psrs{hc}")
                    nc.tensor.matmul(rs[0:64, :], ones_sb[:, 0:64],
                                     p2_all[:, 4 * hc:4 * (hc + 1), 0, :],
                                     start=True, stop=True)
                    nc.tensor.matmul(rs[64:128, :], ones_sb[:, 0:64],
                                     p2_all[:, 4 * hc:4 * (hc + 1), 1, :],
                                     start=True, stop=True)
                    av4 = psAV.tile([P, 4 * P], f32, tag="av", name=f"av{hc}")
                    for hp4 in range(4):
                        hp = 4 * hc + hp4
                        rec = rec_pool.tile([P, P], f32, tag="rec")
                        nc.vector.reciprocal_approx_fast(
                            out=rec[:], in_=rs[:, hp4 * P:(hp4 + 1) * P])
                        av = av4[:, hp4 * P:(hp4 + 1) * P]
                        nc.tensor.matmul(av[0:64, :],
                                         v_sb[:, c, (2 * hp) * DK:(2 * hp + 1) * DK],
                                         p2_all[:, hp, 0, :],
                                         start=True, stop=True)
                        nc.tensor.matmul(av[64:128, :],
                                         v_sb[:, c, (2 * hp + 1) * DK:(2 * hp + 2) * DK],
                                         p2_all[:, hp, 1, :],
                                         start=True, stop=True)
                        nc.vector.tensor_mul(oT_sb[:, hp, tsl], av[:], rec[:])

            # ---- V projection for chunk 0 ----
            v_chunk(0)

            # ---- main loop: per chunk, scores interleave with the previous
            # chunk's Y projection halves (hides the exp chain + psum reuse),
            # then R, then the next chunk's V projection. All (128,128) except
            # R: 2 PE tiling-mode transitions per chunk.
            for c in range(NC_CHUNKS):
                scores_group(c, (0, 1))
                scores_group(c, (2, 3))
                v_half(1, 0) if c == 0 else y_half(c - 1, 0)
                scores_group(c, (4, 5))
                scores_group(c, (6, 7))
                v_half(1, 1) if c == 0 else y_half(c - 1, 1)
                r_phase(c)
                if 1 <= c < NC_CHUNKS - 1:
                    v_chunk(c + 1)

            y_chunk_tail(NC_CHUNKS - 1)

    nc.compile()
    return nc


def _get_program(compute):
    if compute not in _cache:
        _cache[compute] = _build_program(compute)
    return _cache[compute]


DEFAULT_COMPUTE = "bf16"


def kernel(
    query,
    key,
    value,
    Wq,
    bq,
    Wk,
    bk,
    Wv,
    bv,
    Wo,
    bo,
    _compute=DEFAULT_COMPUTE,
    _trace=False,
):
    from concourse.bass_utils import run_bass_kernel_spmd

    nc = _get_program(_compute)
    if _compute == "bf16":
        import ml_dtypes

        npdt = ml_dtypes.bfloat16
    else:
        npdt = {"f32": np.float32, "f16": np.float16}[_compute]

    def pre_w(w):
        # [din, dout] -> [m, p, ko, c] tiles so each m-tile DMAs contiguously
        return np.ascontiguousarray(
            np.asarray(w, np.float32)
            .reshape(KO, P, MO, P)
            .transpose(2, 1, 0, 3)
            .astype(npdt)
        )

    def pre_x(x2, rows):
        # [tok, din] slice -> [p, ko, t] (partition-major, 4KB half-lines)
        return np.ascontiguousarray(
            x2[rows].T.reshape(KO, P, T).transpose(1, 0, 2).astype(npdt)
        )

    q2 = np.asarray(query, np.float32).reshape(B * S, D)
    k2 = np.asarray(key, np.float32).reshape(B * S, D)
    v2 = np.asarray(value, np.float32).reshape(B * S, D)
    # bv folds through attention (rows sum to 1): y = attn Wo + (bv Wo + bo)
    bo_eff = (np.asarray(bv, np.float64) @ np.asarray(Wo, np.float64)
              + np.asarray(bo, np.float64)).astype(np.float32)
    shared = {
        "wq": pre_w(Wq),
        "wk": pre_w(Wk),
        "wv": np.ascontiguousarray(np.asarray(Wv, np.float32).astype(npdt)),
        "wo": np.ascontiguousarray(np.asarray(Wo, np.float32).astype(npdt)),
        "bq": np.ascontiguousarray(np.asarray(bq, np.float32).reshape(MO, P).T),
        "bo": np.ascontiguousarray(bo_eff.astype(npdt)),
    }
    in_maps = []
    for c in range(N_CORES):
        rows = slice(c * T, (c + 1) * T)
        in_maps.append(
            {
                "xq": pre_x(q2, rows),
                "xk": pre_x(k2, rows),
                "xv": pre_x(v2, rows),
                **shared,
            }
        )

    kwargs = {}
    if _trace:
        kwargs = {"trace": True}
    res = run_bass_kernel_spmd(nc, in_maps, core_ids=list(range(N_CORES)), **kwargs)
    y = np.concatenate(
        [res.results[c]["y"].astype(np.float32) for c in range(N_CORES)], axis=0
    )
    out = y.reshape(B, S, D)
    if _trace:
        return out, res
    return out
